# revision 7
# baseline (speedup 1.0000x reference)
"""Trainium2 Bass kernel for nn_PhiCell: y_t = Phi(x_t*k - s_{t-1}) + s_{t-1}.

Phi is the identity, so the scan step is o = (x - s) + s, which is exactly x
in real arithmetic; in fp32 it deviates from x by at most a couple of ulps
and the deviation does not accumulate (the state is overwritten with ~x each
step). The kernel therefore computes the memory-roofline equivalent
y = x * k elementwise, sharded across 8 NeuronCores, and derives the final
state from the last output element. The first element is fixed up exactly on
the host using the provided initial state (one scalar op).

Raw Bass (no Tile): the TRN2 compute/DMA ISA encodes a single sync-wait slot
per instruction, so each chunk's load gets a private semaphore, computes
count on one semaphore, and stores count on another — every instruction
waits on at most one condition.

Layout per core: x is [128, COLS] fp32, k is [128, 1] (the scalar
replicated across partitions, so tensor_scalar can read it per-partition);
y is [128, COLS].
"""

from contextlib import ExitStack

import numpy as np

import concourse.mybir as mybir
from concourse.bass import Bass
from concourse.bass_utils import run_bass_kernel_spmd

T = 4194304
N_CORES = 8
PER_CORE = T // N_CORES          # 524288 elements, 2 MiB per core
P = 128                          # SBUF partitions
COLS = PER_CORE // P             # 4096 fp32 per partition
CHUNK = 1024                     # tile width in fp32 columns
F32 = mybir.dt.float32
_cache: dict = {}


def _build_nc(chunk: int = CHUNK) -> Bass:
    nchunks = COLS // chunk
    nc = Bass()
    x = nc.declare_dram_parameter("x", [P, COLS], F32, isOutput=False)
    k = nc.declare_dram_parameter("k", [P, 1], F32, isOutput=False)
    y = nc.declare_dram_parameter("y", [P, COLS], F32, isOutput=True)

    with ExitStack() as st:
        block = st.enter_context(nc.Block())
        k_sem = st.enter_context(nc.semaphore("k_sem"))
        load_sems = [
            st.enter_context(nc.semaphore(f"load_sem{j}")) for j in range(nchunks)
        ]
        comp_sem = st.enter_context(nc.semaphore("comp_sem"))
        store_sem = st.enter_context(nc.semaphore("store_sem"))
        kt = st.enter_context(nc.sbuf_tensor("kt", [P, 1], F32))
        tiles = [
            st.enter_context(nc.sbuf_tensor(f"t{j}", [P, chunk], F32))
            for j in range(nchunks)
        ]

        @block.sync
        def _(sync):
            sync.dma_start(out=kt[:, :], in_=k[:, :]).then_inc(k_sem, 16)
            for j in range(nchunks):
                sync.dma_start(
                    out=tiles[j][:, :],
                    in_=x[:, j * chunk : (j + 1) * chunk],
                ).then_inc(load_sems[j], 16)
            sync.wait_ge(store_sem, 16 * nchunks)

        @block.vector
        def _(vector):
            vector.wait_ge(k_sem, 16)
            for j in range(nchunks):
                vector.wait_ge(load_sems[j], 16)
                vector.tensor_scalar_mul(
                    out=tiles[j][:, :], in0=tiles[j][:, :], scalar1=kt[:, 0:1]
                ).then_inc(comp_sem, 1)

        @block.scalar
        def _(scalar):
            for j in range(nchunks):
                scalar.wait_ge(comp_sem, j + 1)
                scalar.dma_start(
                    out=y[:, j * chunk : (j + 1) * chunk], in_=tiles[j][:, :]
                ).then_inc(store_sem, 16)

    return nc


def _get_nc() -> Bass:
    if "nc" not in _cache:
        _cache["nc"] = _build_nc()
    return _cache["nc"]


def _run(x_flat: np.ndarray, k_val: np.float32, trace: bool = False):
    nc = _get_nc()
    shards = x_flat.reshape(N_CORES, P, COLS)
    kcol = np.full((P, 1), k_val, dtype=np.float32)
    in_maps = [
        {"x": np.ascontiguousarray(shards[i]), "k": kcol} for i in range(N_CORES)
    ]
    res = run_bass_kernel_spmd(nc, in_maps, list(range(N_CORES)), trace=trace)
    out = np.concatenate([res.results[i]["y"].reshape(-1) for i in range(N_CORES)])
    return out, res


def kernel(inputs: np.ndarray, state: np.ndarray, kernel: np.ndarray):
    x = np.ascontiguousarray(np.asarray(inputs, dtype=np.float32)).reshape(-1)
    k_val = np.asarray(kernel, dtype=np.float32).reshape(-1)[0]
    s0 = np.asarray(state, dtype=np.float32).reshape(-1)[0]

    out_flat, _ = _run(x, k_val)
    out = out_flat.reshape(1, T)

    # Element 0 is the only one whose state operand (s0) isn't ~equal to the
    # element itself; reproduce the reference's exact fp32 arithmetic for it.
    scaled0 = np.float32(x[0] * k_val)
    out[0, 0] = np.float32(np.float32(scaled0 - s0) + s0)

    new_state = np.array([[out[0, -1]]], dtype=np.float32)
    return out, new_state


# revision 8
# speedup vs baseline: 1.0427x; 1.0427x over previous
"""Trainium2 Bass kernel for nn_PhiCell: y_t = Phi(x_t*k - s_{t-1}) + s_{t-1}.

Phi is the identity, so the scan step is o = (x - s) + s, which is exactly x
in real arithmetic; in fp32 it deviates from x by at most a couple of ulps
and the deviation does not accumulate (the state is overwritten with ~x each
step). The kernel therefore computes the memory-roofline equivalent
y = x * k elementwise, sharded across 8 NeuronCores, and derives the final
state from the last output element. The first element is fixed up exactly on
the host using the provided initial state (one scalar op).

Raw Bass (no Tile): the TRN2 compute/DMA ISA encodes a single sync-wait slot
per instruction, so each chunk's load gets a private semaphore, computes
count on one semaphore, and stores count on another — every instruction
waits on at most one condition.

Layout per core: x is [128, COLS+1] fp32 whose column 0 carries k
(replicated across partitions by the host) so the scalar arrives with
chunk 0's load — no separate k DMA on the load ring. Compute reads it as a
per-partition tensor_scalar operand; computes run in DVE program order, so
chunks 1+ need no extra wait for it. y is [128, COLS].
"""

from contextlib import ExitStack

import numpy as np

import concourse.mybir as mybir
from concourse.bass import Bass
from concourse.bass_utils import run_bass_kernel_spmd

T = 4194304
N_CORES = 8
PER_CORE = T // N_CORES          # 524288 elements, 2 MiB per core
P = 128                          # SBUF partitions
COLS = PER_CORE // P             # 4096 fp32 per partition
CHUNK = 1024                     # tile width in fp32 columns
F32 = mybir.dt.float32
_cache: dict = {}


def _build_nc(chunk: int = CHUNK) -> Bass:
    nchunks = COLS // chunk
    nc = Bass()
    x = nc.declare_dram_parameter("x", [P, COLS + 1], F32, isOutput=False)
    y = nc.declare_dram_parameter("y", [P, COLS], F32, isOutput=True)

    with ExitStack() as st:
        block = st.enter_context(nc.Block())
        load_sems = [
            st.enter_context(nc.semaphore(f"load_sem{j}")) for j in range(nchunks)
        ]
        comp_sem = st.enter_context(nc.semaphore("comp_sem"))
        store_sem = st.enter_context(nc.semaphore("store_sem"))
        t0 = st.enter_context(nc.sbuf_tensor("t0", [P, chunk + 1], F32))
        tiles = [t0] + [
            st.enter_context(nc.sbuf_tensor(f"t{j}", [P, chunk], F32))
            for j in range(1, nchunks)
        ]

        @block.sync
        def _(sync):
            sync.dma_start(out=tiles[0][:, :], in_=x[:, 0 : chunk + 1]).then_inc(
                load_sems[0], 16
            )
            for j in range(1, nchunks):
                sync.dma_start(
                    out=tiles[j][:, :],
                    in_=x[:, 1 + j * chunk : 1 + (j + 1) * chunk],
                ).then_inc(load_sems[j], 16)
            sync.wait_ge(store_sem, 16 * nchunks)

        @block.vector
        def _(vector):
            kt = tiles[0][:, 0:1]
            for j in range(nchunks):
                vector.wait_ge(load_sems[j], 16)
                d = tiles[j][:, 1:] if j == 0 else tiles[j][:, :]
                vector.tensor_scalar_mul(out=d, in0=d, scalar1=kt).then_inc(comp_sem, 1)

        @block.scalar
        def _(scalar):
            for j in range(nchunks):
                scalar.wait_ge(comp_sem, j + 1)
                src = tiles[j][:, 1:] if j == 0 else tiles[j][:, :]
                scalar.dma_start(
                    out=y[:, j * chunk : (j + 1) * chunk], in_=src
                ).then_inc(store_sem, 16)

    return nc


def _get_nc() -> Bass:
    if "nc" not in _cache:
        _cache["nc"] = _build_nc()
    return _cache["nc"]


def _run(x_flat: np.ndarray, k_val: np.float32, trace: bool = False):
    nc = _get_nc()
    shards = x_flat.reshape(N_CORES, P, COLS)
    kcol = np.full((P, 1), k_val, dtype=np.float32)
    in_maps = [
        {"x": np.ascontiguousarray(np.concatenate([kcol, shards[i]], axis=1))}
        for i in range(N_CORES)
    ]
    res = run_bass_kernel_spmd(nc, in_maps, list(range(N_CORES)), trace=trace)
    out = np.concatenate([res.results[i]["y"].reshape(-1) for i in range(N_CORES)])
    return out, res


def kernel(inputs: np.ndarray, state: np.ndarray, kernel: np.ndarray):
    x = np.ascontiguousarray(np.asarray(inputs, dtype=np.float32)).reshape(-1)
    k_val = np.asarray(kernel, dtype=np.float32).reshape(-1)[0]
    s0 = np.asarray(state, dtype=np.float32).reshape(-1)[0]

    out_flat, _ = _run(x, k_val)
    out = out_flat.reshape(1, T)

    # Element 0 is the only one whose state operand (s0) isn't ~equal to the
    # element itself; reproduce the reference's exact fp32 arithmetic for it.
    scaled0 = np.float32(x[0] * k_val)
    out[0, 0] = np.float32(np.float32(scaled0 - s0) + s0)

    new_state = np.array([[out[0, -1]]], dtype=np.float32)
    return out, new_state
